# revision 22
# baseline (speedup 1.0000x reference)
"""Trainium2 Bass kernel for per-atom MLP grouped GEMM (moe_routing).

Problem: e[s,a] = MLP_a(g[s,a,:]) for S=2000 structs, A=1000 atoms,
each atom owning a tiny 5->32->32->1 tanh MLP.

Strategy:
  - Shard atoms across 8 cores (125 atoms/core, padded to 128).
  - Per core: 8 groups of 16 atoms; each group's 16 atoms are mapped to a
    4x4 grid of 32x32 PE sub-tiles (tile_position packing) so 16 tiny
    matmuls run concurrently on the 128x128 systolic array.
  - Layer biases: b1 folded in as an extra contraction row (ones row in g),
    b2 via a K=1 ones-matmul accumulated into the same PSUM group,
    b3 added on the host.
  - tanh on the scalar engine (ACT), reading [128, 2048] PSUM spans.
  - bf16 operands for the matmuls (PE streams 1 col/cycle), fp32 PSUM accum.
"""

import os
import sys

sys.path.insert(0, "/opt/trn_rl_repo")

import numpy as np
import ml_dtypes
from contextlib import ExitStack

import concourse.bass as bass
import concourse.tile as tile
from concourse import bacc, mybir
from concourse.bass_utils import run_bass_kernel_spmd

# ---- problem constants (hardcoded; kernel.py must be self-contained) ----
S, A, I, H = 2000, 1000, 5, 32
N_CORES = 8
A_PC = 128          # atoms per core, padded from 125
A_REAL = 125        # real atoms per core
G = 8               # atom groups per core (16 atoms each)
NS = 512            # struct tile (free dim per matmul)
ST = 4              # struct tiles (S padded to 2048)
S_PAD = NS * ST
K1 = 6              # L1 contraction: 5 inputs + 1 ones row (bias)
WCOLS = 512         # per-group weight cols: 128 W1 | 128 W2 | 128 b2 | 4x32 W3

BF16 = mybir.dt.bfloat16
FP32 = mybir.dt.float32
FP32R = mybir.dt.float32r
# matmul operand dtype: BF16 (default) or FP32R (TF32-like, fp32 storage)
MM_DTYPE = BF16

_cached = {}


def _build_program(repeat=1, detect_races=True, sim_safe=False, mm_dtype=None):
    """Build the single-core SPMD bass program (same for all 8 cores).
    repeat>1 re-runs the whole computation (for marginal-time benchmarking).
    detect_races=False relaxes CoreSim's stale-read checker (the e-copy
    intentionally reads garbage rows of a reused PSUM slot)."""
    if mm_dtype is None:
        mm_dtype = MM_DTYPE
    # assisted tanh: DVE copies PSUM fp32 -> SBUF bf16, ACT runs tanh
    # bf16->bf16 (2x ScalarE rate) instead of reading PSUM directly.
    # Per-layer char: '0' never, '1' always, '2' odd units only.
    assist_cfg = os.environ.get("BASS_ASSIST", "00")

    def use_assist(layer, u):
        c = assist_cfg[layer]
        return c == "1" or (c == "2" and u % 2 == 1)
    nc = bacc.Bacc(
        "TRN2",
        target_bir_lowering=False,
        debug=False,
        detect_race_conditions=detect_races,
    )
    gp = nc.dram_tensor("gp", [G, 4, K1, ST, 4, NS], mm_dtype, kind="ExternalInput").ap()
    wp = nc.dram_tensor("wp", [G, 128, WCOLS], mm_dtype, kind="ExternalInput").ap()
    eo = nc.dram_tensor("eo", [G, ST, 4, 4, NS], FP32, kind="ExternalOutput").ap()

    with tile.TileContext(nc) as tc:
        with ExitStack() as ctx:
            wpool = ctx.enter_context(tc.tile_pool(name="w", bufs=1))
            opool = ctx.enter_context(tc.tile_pool(name="ones", bufs=1))
            gpool = ctx.enter_context(tc.tile_pool(name="g", bufs=2))
            hpool = ctx.enter_context(tc.tile_pool(name="h", bufs=4))
            epool = ctx.enter_context(tc.tile_pool(name="e", bufs=3))
            pspool = ctx.enter_context(
                tc.tile_pool(name="ps", bufs=1, space="PSUM")
            )

            # persistent weights for all groups: [128, G*400]
            wt = wpool.tile([128, G * WCOLS], mm_dtype)
            for g in range(G):
                nc.sync.dma_start(wt[:, g * WCOLS : (g + 1) * WCOLS], wp[g])

            ones = opool.tile([128, NS], mm_dtype)
            nc.gpsimd.memset(ones[:], 1.0)

            FW = ST * 2112  # per-stile free blocks padded to 2112
            Tanh = mybir.ActivationFunctionType.Tanh
            gts = {}

            def load_group(g, occ):
                # one g tile per group occurrence; per-i DMAs keep partition
                # APs 2D (multi-partition-dim write APs break Tile deps)
                gt = gpool.tile([128, FW], mm_dtype, tag="gt", name=f"gt{occ}")
                gtr = gt.rearrange("p (s f) -> p s f", f=2112)
                for i in range(4):
                    nc.sync.dma_start(
                        gtr[32 * i : 32 * i + K1, :, 0:2048], gp[g, i]
                    )
                gts[occ] = gt

            def stage_front(u):
                """L1 matmuls + tanh1 for unit u = (g, st)."""
                occ, st = divmod(u, ST)
                g = occ % G
                wg = g * WCOLS
                gt = gts[u // ST]
                # ---- L1: 16 tiles (i,j): K=6, M=32, N=512 ----
                ps1 = pspool.tile([128, 2048], FP32, tag="ps1", name=f"ps1_{u}")
                for j in range(4):
                    for i in range(4):
                        nc.tensor.matmul(
                            ps1[32 * j : 32 * j + 32, i * NS : (i + 1) * NS],
                            lhsT=wt[32 * i : 32 * i + K1, wg + j * 32 : wg + j * 32 + 32],
                            rhs=gt[32 * i : 32 * i + K1, st * 2112 + j * NS : st * 2112 + (j + 1) * NS],
                            start=True,
                            stop=True,
                            tile_position=(32 * i, 32 * j),
                        )
                h1 = hpool.tile([128, 2048], mm_dtype, tag="h", name=f"h1_{u}")
                if use_assist(0, u):
                    z1 = hpool.tile([128, 2048], mm_dtype, tag="z", name=f"z1_{u}")
                    nc.vector.tensor_copy(z1[:], ps1[:])
                    nc.scalar.activation(h1[:], z1[:], Tanh)
                else:
                    nc.scalar.activation(h1[:], ps1[:], Tanh)
                return h1

            def stage_back(u, h1):
                """L2 + tanh2 + L3 + e-copy + e-DMA for unit u."""
                occ, st = divmod(u, ST)
                g = occ % G
                wg = g * WCOLS
                # ---- L2: 16 tiles (j,i): bias K=1 then W2 K=32 ----
                # bias/data pairs adjacent (PSUM accumulation groups are
                # tracked per tile_position; the pair must share one)
                ps2 = pspool.tile([128, 2048], FP32, tag="ps2", name=f"ps2_{u}")
                for i in range(4):
                    for j in range(4):
                        nc.tensor.matmul(
                            ps2[32 * i : 32 * i + 32, j * NS : (j + 1) * NS],
                            lhsT=wt[32 * j : 32 * j + 1, wg + 256 + i * 32 : wg + 256 + i * 32 + 32],
                            rhs=ones[32 * j : 32 * j + 1, :],
                            start=True,
                            stop=False,
                            tile_position=(32 * j, 32 * i),
                        )
                        nc.tensor.matmul(
                            ps2[32 * i : 32 * i + 32, j * NS : (j + 1) * NS],
                            lhsT=wt[32 * j : 32 * j + 32, wg + 128 + i * 32 : wg + 128 + i * 32 + 32],
                            rhs=h1[32 * j : 32 * j + 32, i * NS : (i + 1) * NS],
                            start=False,
                            stop=True,
                            tile_position=(32 * j, 32 * i),
                        )
                h2 = hpool.tile([128, 2048], mm_dtype, tag="h", name=f"h2_{u}")
                if use_assist(1, u):
                    z2 = hpool.tile([128, 2048], mm_dtype, tag="z", name=f"z2_{u}")
                    nc.vector.tensor_copy(z2[:], ps2[:])
                    nc.scalar.activation(h2[:], z2[:], Tanh)
                else:
                    nc.scalar.activation(h2[:], ps2[:], Tanh)

                # ---- L3: 4 dense col-tiled MMs: K=128, M=32 (4 real +
                # 28 zero-padded lhsT cols; M=4 outputs don't materialize
                # on HW), N=512 ----
                # MM i reads h2[:, i*NS:(i+1)*NS] = atoms (p, i), p=0..3,
                # with feature k at partition 32p+k; lhsT col m holds
                # W3[atom(m, i)] in rows 32m+k, so partition 32i+m gets
                # e[atom(m, i)]. Written back into ps2 (bank 0) after T2
                # consumed it, keeping the PSUM footprint at 8 banks.
                for i in range(4):
                    nc.tensor.matmul(
                        ps2[32 * i : 32 * i + 32, 0:NS],
                        lhsT=wt[:, wg + 384 + 32 * i : wg + 384 + 32 * i + 32],
                        rhs=h2[:, i * NS : (i + 1) * NS],
                        start=True,
                        stop=True,
                        tile_position=(0, 32 * i),
                    )
                # DVE-copy e rows to SBUF (DMA cannot read PSUM).
                # The full-width copy also reads rows {32i+4..} holding
                # stale z2 values (harmless; only rows 32i+j ship out) -
                # sim_safe does 4 exact block copies for CoreSim's
                # stale-read checker.
                et = epool.tile([128, NS], FP32, tag="e", name=f"et_{u}")
                if sim_safe:
                    for i in range(4):
                        nc.vector.tensor_copy(
                            et[32 * i : 32 * i + 4, :],
                            ps2[32 * i : 32 * i + 4, 0:NS],
                        )
                elif os.environ.get("BASS_ECOPY", "v") == "s":
                    nc.scalar.copy(et[:], ps2[:, 0:NS])
                else:
                    nc.vector.tensor_copy(et[:], ps2[:, 0:NS])
                # per-i DMAs: single-partition-dim APs only (multi-partition
                # -dim APs with inner count > 1 read out of the tile)
                for i in range(4):
                    nc.sync.dma_start(
                        eo[g, st, i], et[32 * i : 32 * i + 4, :]
                    )

            # Software-pipelined emission: unit u's front (L1+T1) is emitted
            # before unit u-1's back (L2+T2+L3), so ACT program order is
            # T1(u), T2(u-1), T1(u+1), T2(u), ... -- the PE work for T2(u-1)
            # and T1(u+1) is always one ACT-instruction ahead, keeping ACT
            # busy back-to-back. ps1 tiles share one 4-bank buffer (tag
            # rotation bufs=1), ps2 the other 4 banks.
            n_units = repeat * G * ST
            h1s = {}
            for u in range(n_units):
                if u % ST == 0:
                    load_group((u // ST) % G, u // ST)
                h1s[u] = stage_front(u)
                if u > 0:
                    stage_back(u - 1, h1s.pop(u - 1))
            stage_back(n_units - 1, h1s.pop(n_units - 1))

    nc.compile()
    return nc


def _pack_inputs(g, W1, b1, W2, b2, W3):
    """Pack full inputs into per-core DRAM layouts (vectorized)."""
    bf = ml_dtypes.bfloat16 if MM_DTYPE == BF16 else np.float32
    # pad atoms to N_CORES*A_PC with zeros
    A_pad = N_CORES * A_PC

    def pad_atoms(x):
        out = np.zeros((N_CORES, A_PC) + x.shape[1:], dtype=np.float32)
        xr = x.reshape(N_CORES, A_REAL, *x.shape[1:])
        out[:, :A_REAL] = xr
        return out

    W1p = pad_atoms(W1)          # [C, 128, 5, 32]
    b1p = pad_atoms(b1)          # [C, 128, 32]
    W2p = pad_atoms(W2)          # [C, 128, 32, 32]
    b2p = pad_atoms(b2)          # [C, 128, 32]
    W3p = pad_atoms(W3)[..., 0]  # [C, 128, 32]

    # g: [S, A, I] -> per core [S_PAD, 128, I] -> gp[grp, i, k, st, j, s]
    gpad = np.zeros((S_PAD, N_CORES, A_PC, I), dtype=np.float32)
    gpad[:S, :, :A_REAL] = g.reshape(S, N_CORES, A_REAL, I).astype(np.float32)

    in_maps = []
    for c in range(N_CORES):
        # ---- gp ----
        gc = gpad[:, c]                                   # [S_PAD, 128, 5]
        gc = gc.reshape(ST, NS, G, 4, 4, I)               # [st, s, grp, i, j, k]
        gp = np.zeros((G, 4, K1, ST, 4, NS), dtype=bf)
        gp[:, :, :I] = gc.transpose(2, 3, 5, 0, 4, 1).astype(bf)
        gp[:, :, I] = bf(1.0)                             # ones row for b1

        # ---- wp ----
        wp = np.zeros((G, 128, WCOLS), dtype=bf)
        wv = wp.reshape(G, 4, 32, WCOLS)                  # rows (blk, k)
        # W1 lhsT: rows 32i+k (k<5: W1, k=5: b1), cols j*32+h
        w1c = W1p[c].reshape(G, 4, 4, I, H)               # [grp, i, j, k, h]
        wv[:, :, :I, 0:128] = w1c.transpose(0, 1, 3, 2, 4).reshape(G, 4, I, 128).astype(bf)
        b1c = b1p[c].reshape(G, 4, 4, H)                  # [grp, i, j, h]
        wv[:, :, I, 0:128] = b1c.reshape(G, 4, 128).astype(bf)
        # W2 lhsT: rows 32j+k, cols 128 + i*32+h
        w2c = W2p[c].reshape(G, 4, 4, H, H)               # [grp, i, j, k, h]
        wv[:, :, :, 128:256] = w2c.transpose(0, 2, 3, 1, 4).reshape(G, 4, 32, 128).astype(bf)
        # b2 lhsT: row 32j, cols 256 + i*32+h
        b2c = b2p[c].reshape(G, 4, 4, H)                  # [grp, i, j, h]
        wv[:, :, 0, 256:384] = b2c.transpose(0, 2, 1, 3).reshape(G, 4, 128).astype(bf)
        # W3 dense lhsT for L3 MM i: col 384+32i+m (m<4) holds W3[atom(m, i)]
        # in rows 32m+k (matching h2's atom-(p, i) partition layout); cols
        # m>=4 stay zero (M=32 padding).
        w3c = W3p[c].reshape(G, 4, 4, H)                  # [grp, i, j, k]
        for i in range(4):
            for m in range(4):
                wv[:, m, :, 384 + 32 * i + m] = w3c[:, m, i, :].astype(bf)
        in_maps.append({"gp": gp, "wp": wp})
    return in_maps


def _unpack_outputs(results, b3):
    """Assemble [S, A] output from per-core eo tensors; add b3 on host."""
    out = np.empty((S, A), dtype=np.float32)
    for c in range(N_CORES):
        e = results[c]["eo"]                           # [grp, st, i, m, s]
        # value at (i, m) is e[atom(m, i)] -> [st*s, grp, m, i] -> [S_PAD, 128]
        e = e.transpose(1, 4, 0, 3, 2).reshape(S_PAD, G * 16)
        out[:, c * A_REAL : (c + 1) * A_REAL] = e[:S, :A_REAL]
    out += b3[None, :, 0]
    return out


def _make_runner(nc):
    """Build a reusable jitted SPMD callable (mirrors bass2jax.run_bass_via_pjrt
    but caches the jitted function so repeated calls don't re-trace)."""
    import jax
    from jax.sharding import Mesh, PartitionSpec
    from jax.experimental.shard_map import shard_map
    from concourse import bass2jax
    from concourse.bass2jax import (
        _bass_exec_p,
        install_neuronx_cc_hook,
        partition_id_tensor,
    )

    install_neuronx_cc_hook()

    partition_name = nc.partition_id_tensor.name if nc.partition_id_tensor else None
    in_names, out_names, out_avals = [], [], []
    for alloc in nc.m.functions[0].allocations:
        if not isinstance(alloc, mybir.MemoryLocationSet):
            continue
        name = alloc.memorylocations[0].name
        if alloc.kind == "ExternalInput":
            if name == partition_name:
                continue
            in_names.append(name)
        elif alloc.kind == "ExternalOutput":
            out_names.append(name)
            out_avals.append(
                jax.core.ShapedArray(
                    tuple(alloc.tensor_shape), mybir.dt.np(alloc.dtype)
                )
            )
    n_params = len(in_names)
    n_outs = len(out_avals)
    all_names = in_names + out_names
    if partition_name is not None:
        all_names = all_names + [partition_name]

    def _body(*args):
        operands = list(args)
        if partition_name is not None:
            operands.append(partition_id_tensor())
        outs = _bass_exec_p.bind(
            *operands,
            out_avals=tuple(out_avals),
            in_names=tuple(all_names),
            out_names=tuple(out_names),
            lowering_input_output_aliases=(),
            sim_require_finite=True,
            sim_require_nnan=True,
            nc=nc,
        )
        return tuple(outs)

    devices = jax.devices()[:N_CORES]
    mesh = Mesh(np.asarray(devices), ("core",))
    from jax.sharding import NamedSharding
    nspec = NamedSharding(mesh, PartitionSpec("core"))
    in_specs = (PartitionSpec("core"),) * (n_params + n_outs)
    out_specs = (PartitionSpec("core"),) * n_outs
    sharded = jax.jit(
        shard_map(_body, mesh=mesh, in_specs=in_specs, out_specs=out_specs,
                  check_rep=False),
        keep_unused=True,
    )

    def device_put_inputs(in_maps):
        arrs = [
            jax.device_put(
                np.concatenate([np.asarray(m[name]) for m in in_maps], axis=0),
                nspec,
            )
            for name in in_names
        ]
        # zero output-buffer operands, device-resident, reused (not donated)
        arrs += [
            jax.device_put(
                np.zeros((N_CORES * a.shape[0], *a.shape[1:]), a.dtype), nspec
            )
            for a in out_avals
        ]
        return arrs

    def run_device(concat_in):
        return sharded(*concat_in)

    def run(in_maps):
        out_arrs = sharded(*device_put_inputs(in_maps))
        return [
            {
                name: np.asarray(out_arrs[i]).reshape(
                    N_CORES, *out_avals[i].shape
                )[c]
                for i, name in enumerate(out_names)
            }
            for c in range(N_CORES)
        ], out_arrs

    run.device_put_inputs = device_put_inputs
    run.run_device = run_device
    return run


def get_runner():
    if "run" not in _cached:
        _cached["nc"] = _build_program()
        _cached["run"] = _make_runner(_cached["nc"])
    return _cached["run"]


def kernel(g, W1, b1, W2, b2, W3, b3):
    g = np.asarray(g, dtype=np.float32)
    W1 = np.asarray(W1, dtype=np.float32)
    b1 = np.asarray(b1, dtype=np.float32)
    W2 = np.asarray(W2, dtype=np.float32)
    b2 = np.asarray(b2, dtype=np.float32)
    W3 = np.asarray(W3, dtype=np.float32)
    b3 = np.asarray(b3, dtype=np.float32)

    run = get_runner()
    in_maps = _pack_inputs(g, W1, b1, W2, b2, W3)
    results, _ = run(in_maps)
    return _unpack_outputs(results, b3)


if __name__ == "__main__":
    # quick self-test against a small numpy model
    rng = np.random.default_rng(0)
    g = rng.standard_normal((S, A, I), dtype=np.float32)
    W1 = rng.standard_normal((A, I, H), dtype=np.float32) * 0.45
    b1 = rng.standard_normal((A, H), dtype=np.float32) * 0.01
    W2 = rng.standard_normal((A, H, H), dtype=np.float32) * 0.18
    b2 = rng.standard_normal((A, H), dtype=np.float32) * 0.01
    W3 = rng.standard_normal((A, H, 1), dtype=np.float32) * 0.18
    b3 = rng.standard_normal((A, 1), dtype=np.float32) * 0.01
    out = kernel(g, W1, b1, W2, b2, W3, b3)
    h1 = np.tanh(np.einsum("sai,aih->sah", g, W1) + b1[None])
    h2 = np.tanh(np.einsum("sah,aho->sao", h1, W2) + b2[None])
    ref = (np.einsum("sah,aho->sao", h2, W3) + b3[None])[..., 0]
    rel = np.abs(out - ref).max() / np.abs(ref).max()
    print("max rel err:", rel)



# revision 24
# speedup vs baseline: 1.2384x; 1.2384x over previous
"""Trainium2 Bass kernel for per-atom MLP grouped GEMM (moe_routing).

Problem: e[s,a] = MLP_a(g[s,a,:]) for S=2000 structs, A=1000 atoms,
each atom owning a tiny 5->32->32->1 tanh MLP.

Strategy:
  - Shard atoms across 8 cores (125 atoms/core, padded to 128).
  - Per core: 8 groups of 16 atoms; each group's 16 atoms are mapped to a
    4x4 grid of 32x32 PE sub-tiles (tile_position packing) so 16 tiny
    matmuls run concurrently on the 128x128 systolic array.
  - Layer biases: b1 folded in as an extra contraction row (ones row in g),
    b2 via a K=1 ones-matmul accumulated into the same PSUM group,
    b3 added on the host.
  - tanh on the scalar engine (ACT), reading [128, 2048] PSUM spans.
  - bf16 operands for the matmuls (PE streams 1 col/cycle), fp32 PSUM accum.
"""

import os
import sys

sys.path.insert(0, "/opt/trn_rl_repo")

import numpy as np
import ml_dtypes
from contextlib import ExitStack

import concourse.bass as bass
import concourse.tile as tile
from concourse import bacc, mybir
from concourse.bass_utils import run_bass_kernel_spmd

# ---- problem constants (hardcoded; kernel.py must be self-contained) ----
S, A, I, H = 2000, 1000, 5, 32
N_CORES = 8
A_PC = 128          # atoms per core, padded from 125
A_REAL = 125        # real atoms per core
G = 8               # atom groups per core (16 atoms each)
NS = 512            # struct tile (free dim per matmul)
ST = 4              # struct tiles (S padded to 2048)
S_PAD = NS * ST
K1 = 6              # L1 contraction: 5 inputs + 1 ones row (bias)
WCOLS = 512         # per-group weight cols: 128 W1 | 128 W2 | 128 b2 | 4x32 W3

BF16 = mybir.dt.bfloat16
FP32 = mybir.dt.float32
FP32R = mybir.dt.float32r
# matmul operand dtype: BF16 (default) or FP32R (TF32-like, fp32 storage)
MM_DTYPE = BF16

_cached = {}


def _build_program(repeat=1, detect_races=True, sim_safe=False, mm_dtype=None):
    """Build the single-core SPMD bass program (same for all 8 cores).
    repeat>1 re-runs the whole computation (for marginal-time benchmarking).
    detect_races=False relaxes CoreSim's stale-read checker (the e-copy
    intentionally reads garbage rows of a reused PSUM slot)."""
    if mm_dtype is None:
        mm_dtype = MM_DTYPE
    # assisted tanh: DVE copies PSUM fp32 -> SBUF bf16, ACT runs tanh
    # bf16->bf16 (2x ScalarE rate) instead of reading PSUM directly.
    # Per-layer char: '0' never, '1' always, '2' odd units only.
    assist_cfg = os.environ.get("BASS_ASSIST", "00")

    def use_assist(layer, u):
        c = assist_cfg[layer]
        return c == "1" or (c == "2" and u % 2 == 1)
    nc = bacc.Bacc(
        "TRN2",
        target_bir_lowering=False,
        debug=False,
        detect_race_conditions=detect_races,
    )
    gp = nc.dram_tensor("gp", [G, 4, K1, ST, 4, NS], mm_dtype, kind="ExternalInput").ap()
    wp = nc.dram_tensor("wp", [G, 128, WCOLS], mm_dtype, kind="ExternalInput").ap()
    eo = nc.dram_tensor("eo", [G, ST, 4, 4, NS], FP32, kind="ExternalOutput").ap()

    with tile.TileContext(nc) as tc:
        with ExitStack() as ctx:
            wpool = ctx.enter_context(tc.tile_pool(name="w", bufs=1))
            opool = ctx.enter_context(tc.tile_pool(name="ones", bufs=1))
            gpool = ctx.enter_context(tc.tile_pool(name="g", bufs=2))
            hpool = ctx.enter_context(tc.tile_pool(name="h", bufs=4))
            epool = ctx.enter_context(tc.tile_pool(name="e", bufs=3))
            pspool = ctx.enter_context(
                tc.tile_pool(name="ps", bufs=1, space="PSUM")
            )

            # persistent weights for all groups: [128, G*400]
            wt = wpool.tile([128, G * WCOLS], mm_dtype)
            for g in range(G):
                nc.sync.dma_start(wt[:, g * WCOLS : (g + 1) * WCOLS], wp[g])

            ones = opool.tile([128, NS], mm_dtype)
            nc.gpsimd.memset(ones[:], 1.0)

            FW = ST * 2112  # per-stile free blocks padded to 2112
            Tanh = mybir.ActivationFunctionType.Tanh
            gts = {}

            def load_group(g, occ):
                # one g tile per group occurrence; per-i DMAs keep partition
                # APs 2D (multi-partition-dim write APs break Tile deps)
                gt = gpool.tile([128, FW], mm_dtype, tag="gt", name=f"gt{occ}")
                gtr = gt.rearrange("p (s f) -> p s f", f=2112)
                for i in range(4):
                    nc.sync.dma_start(
                        gtr[32 * i : 32 * i + K1, :, 0 : 4 * NS], gp[g, i]
                    )
                gts[occ] = gt

            def stage_front(u):
                """L1 matmuls + tanh1 for unit u = (g, st)."""
                occ, st = divmod(u, ST)
                g = occ % G
                wg = g * WCOLS
                gt = gts[u // ST]
                # ---- L1: 16 tiles (i,j): K=6, M=32, N=512 ----
                ps1 = pspool.tile([128, 4 * NS], FP32, tag="ps1", name=f"ps1_{u}")
                for j in range(4):
                    for i in range(4):
                        nc.tensor.matmul(
                            ps1[32 * j : 32 * j + 32, i * NS : (i + 1) * NS],
                            lhsT=wt[32 * i : 32 * i + K1, wg + j * 32 : wg + j * 32 + 32],
                            rhs=gt[32 * i : 32 * i + K1, st * 2112 + j * NS : st * 2112 + (j + 1) * NS],
                            start=True,
                            stop=True,
                            tile_position=(32 * i, 32 * j),
                        )
                h1 = hpool.tile([128, 4 * NS], mm_dtype, tag="h", name=f"h1_{u}")
                if use_assist(0, u):
                    z1 = hpool.tile([128, 4 * NS], mm_dtype, tag="z", name=f"z1_{u}")
                    nc.vector.tensor_copy(z1[:], ps1[:])
                    nc.scalar.activation(h1[:], z1[:], Tanh)
                else:
                    nc.scalar.activation(h1[:], ps1[:], Tanh)
                return h1

            def stage_back(u, h1):
                """L2 + tanh2 + L3 + e-copy + e-DMA for unit u."""
                occ, st = divmod(u, ST)
                g = occ % G
                wg = g * WCOLS
                # ---- L2: 16 tiles (j,i): bias K=1 then W2 K=32 ----
                # bias/data pairs adjacent (PSUM accumulation groups are
                # tracked per tile_position; the pair must share one)
                ps2 = pspool.tile([128, 4 * NS], FP32, tag="ps2", name=f"ps2_{u}")
                for i in range(4):
                    for j in range(4):
                        nc.tensor.matmul(
                            ps2[32 * i : 32 * i + 32, j * NS : (j + 1) * NS],
                            lhsT=wt[32 * j : 32 * j + 1, wg + 256 + i * 32 : wg + 256 + i * 32 + 32],
                            rhs=ones[32 * j : 32 * j + 1, :],
                            start=True,
                            stop=False,
                            tile_position=(32 * j, 32 * i),
                        )
                        nc.tensor.matmul(
                            ps2[32 * i : 32 * i + 32, j * NS : (j + 1) * NS],
                            lhsT=wt[32 * j : 32 * j + 32, wg + 128 + i * 32 : wg + 128 + i * 32 + 32],
                            rhs=h1[32 * j : 32 * j + 32, i * NS : (i + 1) * NS],
                            start=False,
                            stop=True,
                            tile_position=(32 * j, 32 * i),
                        )
                h2 = hpool.tile([128, 4 * NS], mm_dtype, tag="h", name=f"h2_{u}")
                if use_assist(1, u):
                    z2 = hpool.tile([128, 4 * NS], mm_dtype, tag="z", name=f"z2_{u}")
                    nc.vector.tensor_copy(z2[:], ps2[:])
                    nc.scalar.activation(h2[:], z2[:], Tanh)
                else:
                    nc.scalar.activation(h2[:], ps2[:], Tanh)

                # ---- L3: 4 dense col-tiled MMs: K=128, M=32 (4 real +
                # 28 zero-padded lhsT cols; M=4 outputs don't materialize
                # on HW), N=512 ----
                # MM i reads h2[:, i*NS:(i+1)*NS] = atoms (p, i), p=0..3,
                # with feature k at partition 32p+k; lhsT col m holds
                # W3[atom(m, i)] in rows 32m+k, so partition 32i+m gets
                # e[atom(m, i)]. Written back into ps2 (bank 0) after T2
                # consumed it, keeping the PSUM footprint at 8 banks.
                for i in range(4):
                    nc.tensor.matmul(
                        ps2[32 * i : 32 * i + 32, 0:NS],
                        lhsT=wt[:, wg + 384 + 32 * i : wg + 384 + 32 * i + 32],
                        rhs=h2[:, i * NS : (i + 1) * NS],
                        start=True,
                        stop=True,
                        tile_position=(0, 32 * i),
                    )
                # DVE-copy e rows to SBUF (DMA cannot read PSUM).
                # The full-width copy also reads rows {32i+4..} holding
                # stale z2 values (harmless; only rows 32i+j ship out) -
                # sim_safe does 4 exact block copies for CoreSim's
                # stale-read checker.
                et = epool.tile([128, NS], FP32, tag="e", name=f"et_{u}")
                if sim_safe:
                    for i in range(4):
                        nc.vector.tensor_copy(
                            et[32 * i : 32 * i + 4, :],
                            ps2[32 * i : 32 * i + 4, 0:NS],
                        )
                elif os.environ.get("BASS_ECOPY", "v") == "s":
                    nc.scalar.copy(et[:], ps2[:, 0:NS])
                else:
                    nc.vector.tensor_copy(et[:], ps2[:, 0:NS])
                # per-i DMAs: single-partition-dim APs only (multi-partition
                # -dim APs with inner count > 1 read out of the tile)
                for i in range(4):
                    nc.sync.dma_start(
                        eo[g, st, i], et[32 * i : 32 * i + 4, :]
                    )

            # Software-pipelined emission: unit u's front (L1+T1) is emitted
            # before unit u-1's back (L2+T2+L3), so ACT program order is
            # T1(u), T2(u-1), T1(u+1), T2(u), ... -- the PE work for T2(u-1)
            # and T1(u+1) is always one ACT-instruction ahead, keeping ACT
            # busy back-to-back. ps1 tiles share one 4-bank buffer (tag
            # rotation bufs=1), ps2 the other 4 banks.
            n_units = repeat * G * ST
            h1s = {}
            for u in range(n_units):
                if u % ST == 0:
                    load_group((u // ST) % G, u // ST)
                h1s[u] = stage_front(u)
                if u > 0:
                    stage_back(u - 1, h1s.pop(u - 1))
            stage_back(n_units - 1, h1s.pop(n_units - 1))

    nc.compile()
    return nc


def _pack_inputs(g, W1, b1, W2, b2, W3):
    """Pack full inputs into per-core DRAM layouts (vectorized)."""
    bf = ml_dtypes.bfloat16 if MM_DTYPE == BF16 else np.float32
    # pad atoms to N_CORES*A_PC with zeros
    A_pad = N_CORES * A_PC

    def pad_atoms(x):
        out = np.zeros((N_CORES, A_PC) + x.shape[1:], dtype=np.float32)
        xr = x.reshape(N_CORES, A_REAL, *x.shape[1:])
        out[:, :A_REAL] = xr
        return out

    W1p = pad_atoms(W1)          # [C, 128, 5, 32]
    b1p = pad_atoms(b1)          # [C, 128, 32]
    W2p = pad_atoms(W2)          # [C, 128, 32, 32]
    b2p = pad_atoms(b2)          # [C, 128, 32]
    W3p = pad_atoms(W3)[..., 0]  # [C, 128, 32]

    # g: [S, A, I] -> per core [S_PAD, 128, I] -> gp[grp, i, k, st, j, s]
    gpad = np.zeros((S_PAD, N_CORES, A_PC, I), dtype=np.float32)
    gpad[:S, :, :A_REAL] = g.reshape(S, N_CORES, A_REAL, I).astype(np.float32)

    in_maps = []
    for c in range(N_CORES):
        # ---- gp ----
        gc = gpad[:, c]                                   # [S_PAD, 128, 5]
        gc = gc.reshape(ST, NS, G, 4, 4, I)               # [st, s, grp, i, j, k]
        gp = np.zeros((G, 4, K1, ST, 4, NS), dtype=bf)
        gp[:, :, :I] = gc.transpose(2, 3, 5, 0, 4, 1).astype(bf)
        gp[:, :, I] = bf(1.0)                             # ones row for b1

        # ---- wp ----
        wp = np.zeros((G, 128, WCOLS), dtype=bf)
        wv = wp.reshape(G, 4, 32, WCOLS)                  # rows (blk, k)
        # W1 lhsT: rows 32i+k (k<5: W1, k=5: b1), cols j*32+h
        w1c = W1p[c].reshape(G, 4, 4, I, H)               # [grp, i, j, k, h]
        wv[:, :, :I, 0:128] = w1c.transpose(0, 1, 3, 2, 4).reshape(G, 4, I, 128).astype(bf)
        b1c = b1p[c].reshape(G, 4, 4, H)                  # [grp, i, j, h]
        wv[:, :, I, 0:128] = b1c.reshape(G, 4, 128).astype(bf)
        # W2 lhsT: rows 32j+k, cols 128 + i*32+h
        w2c = W2p[c].reshape(G, 4, 4, H, H)               # [grp, i, j, k, h]
        wv[:, :, :, 128:256] = w2c.transpose(0, 2, 3, 1, 4).reshape(G, 4, 32, 128).astype(bf)
        # b2 lhsT: row 32j, cols 256 + i*32+h
        b2c = b2p[c].reshape(G, 4, 4, H)                  # [grp, i, j, h]
        wv[:, :, 0, 256:384] = b2c.transpose(0, 2, 1, 3).reshape(G, 4, 128).astype(bf)
        # W3 dense lhsT for L3 MM i: col 384+32i+m (m<4) holds W3[atom(m, i)]
        # in rows 32m+k (matching h2's atom-(p, i) partition layout); cols
        # m>=4 stay zero (M=32 padding).
        w3c = W3p[c].reshape(G, 4, 4, H)                  # [grp, i, j, k]
        for i in range(4):
            for m in range(4):
                wv[:, m, :, 384 + 32 * i + m] = w3c[:, m, i, :].astype(bf)
        in_maps.append({"gp": gp, "wp": wp})
    return in_maps


def _unpack_outputs(results, b3):
    """Assemble [S, A] output from per-core eo tensors; add b3 on host."""
    out = np.empty((S, A), dtype=np.float32)
    for c in range(N_CORES):
        e = results[c]["eo"]                           # [grp, st, i, m, s]
        # value at (i, m) is e[atom(m, i)] -> [st*s, grp, m, i] -> [S_PAD, 128]
        e = e.transpose(1, 4, 0, 3, 2).reshape(S_PAD, G * 16)
        out[:, c * A_REAL : (c + 1) * A_REAL] = e[:S, :A_REAL]
    out += b3[None, :, 0]
    return out


def _make_runner(nc):
    """Build a reusable jitted SPMD callable (mirrors bass2jax.run_bass_via_pjrt
    but caches the jitted function so repeated calls don't re-trace)."""
    import jax
    from jax.sharding import Mesh, PartitionSpec
    from jax.experimental.shard_map import shard_map
    from concourse import bass2jax
    from concourse.bass2jax import (
        _bass_exec_p,
        install_neuronx_cc_hook,
        partition_id_tensor,
    )

    install_neuronx_cc_hook()

    partition_name = nc.partition_id_tensor.name if nc.partition_id_tensor else None
    in_names, out_names, out_avals = [], [], []
    for alloc in nc.m.functions[0].allocations:
        if not isinstance(alloc, mybir.MemoryLocationSet):
            continue
        name = alloc.memorylocations[0].name
        if alloc.kind == "ExternalInput":
            if name == partition_name:
                continue
            in_names.append(name)
        elif alloc.kind == "ExternalOutput":
            out_names.append(name)
            out_avals.append(
                jax.core.ShapedArray(
                    tuple(alloc.tensor_shape), mybir.dt.np(alloc.dtype)
                )
            )
    n_params = len(in_names)
    n_outs = len(out_avals)
    all_names = in_names + out_names
    if partition_name is not None:
        all_names = all_names + [partition_name]

    def _body(*args):
        operands = list(args)
        if partition_name is not None:
            operands.append(partition_id_tensor())
        outs = _bass_exec_p.bind(
            *operands,
            out_avals=tuple(out_avals),
            in_names=tuple(all_names),
            out_names=tuple(out_names),
            lowering_input_output_aliases=(),
            sim_require_finite=True,
            sim_require_nnan=True,
            nc=nc,
        )
        return tuple(outs)

    devices = jax.devices()[:N_CORES]
    mesh = Mesh(np.asarray(devices), ("core",))
    from jax.sharding import NamedSharding
    nspec = NamedSharding(mesh, PartitionSpec("core"))
    in_specs = (PartitionSpec("core"),) * (n_params + n_outs)
    out_specs = (PartitionSpec("core"),) * n_outs
    sharded = jax.jit(
        shard_map(_body, mesh=mesh, in_specs=in_specs, out_specs=out_specs,
                  check_rep=False),
        keep_unused=True,
    )

    def device_put_inputs(in_maps):
        arrs = [
            jax.device_put(
                np.concatenate([np.asarray(m[name]) for m in in_maps], axis=0),
                nspec,
            )
            for name in in_names
        ]
        # zero output-buffer operands, device-resident, reused (not donated)
        arrs += [
            jax.device_put(
                np.zeros((N_CORES * a.shape[0], *a.shape[1:]), a.dtype), nspec
            )
            for a in out_avals
        ]
        return arrs

    def run_device(concat_in):
        return sharded(*concat_in)

    def run(in_maps):
        out_arrs = sharded(*device_put_inputs(in_maps))
        return [
            {
                name: np.asarray(out_arrs[i]).reshape(
                    N_CORES, *out_avals[i].shape
                )[c]
                for i, name in enumerate(out_names)
            }
            for c in range(N_CORES)
        ], out_arrs

    run.device_put_inputs = device_put_inputs
    run.run_device = run_device
    return run


def get_runner():
    if "run" not in _cached:
        _cached["nc"] = _build_program()
        _cached["run"] = _make_runner(_cached["nc"])
    return _cached["run"]


def kernel(g, W1, b1, W2, b2, W3, b3):
    g = np.asarray(g, dtype=np.float32)
    W1 = np.asarray(W1, dtype=np.float32)
    b1 = np.asarray(b1, dtype=np.float32)
    W2 = np.asarray(W2, dtype=np.float32)
    b2 = np.asarray(b2, dtype=np.float32)
    W3 = np.asarray(W3, dtype=np.float32)
    b3 = np.asarray(b3, dtype=np.float32)

    run = get_runner()
    in_maps = _pack_inputs(g, W1, b1, W2, b2, W3)
    results, _ = run(in_maps)
    return _unpack_outputs(results, b3)


if __name__ == "__main__":
    # quick self-test against a small numpy model
    rng = np.random.default_rng(0)
    g = rng.standard_normal((S, A, I), dtype=np.float32)
    W1 = rng.standard_normal((A, I, H), dtype=np.float32) * 0.45
    b1 = rng.standard_normal((A, H), dtype=np.float32) * 0.01
    W2 = rng.standard_normal((A, H, H), dtype=np.float32) * 0.18
    b2 = rng.standard_normal((A, H), dtype=np.float32) * 0.01
    W3 = rng.standard_normal((A, H, 1), dtype=np.float32) * 0.18
    b3 = rng.standard_normal((A, 1), dtype=np.float32) * 0.01
    out = kernel(g, W1, b1, W2, b2, W3, b3)
    h1 = np.tanh(np.einsum("sai,aih->sah", g, W1) + b1[None])
    h2 = np.tanh(np.einsum("sah,aho->sao", h1, W2) + b2[None])
    ref = (np.einsum("sah,aho->sao", h2, W3) + b3[None])[..., 0]
    rel = np.abs(out - ref).max() / np.abs(ref).max()
    print("max rel err:", rel)

